# revision 26
# baseline (speedup 1.0000x reference)
"""CompositionalAttention Trainium2 kernel (8 NeuronCores, SPMD).

Shapes (hardcoded): query (T=2048, B=2, E=1024), H=16 heads, R=8 rules,
HD=64, VD=128. Output (T, B, E) float32.

Sharding: (batch x t-quarter) -> 8 cores. Core c handles b = c//4 and the
t-slice [tq*512, (tq+1)*512) with tq = c%4, computing ALL heads for that
slice so the output projection needs no cross-core reduction. Each core
returns its exact (512, 1024) slice of the final output.

Algebraic simplification used (verified vs reference to 2.5e-6):
the rule-selection softmax input is
    score[b,h,t,r] = v_q . w_sel + bsc + attn[b,h,t,r,:] . w_vd
and the first two terms are constant in r, so they cancel in the softmax
over r. Wvq/bvq/Wsc[:, :SEL]/bsc never affect the output. Further, with
unnormalized attention A~_r = P~ @ v_r (P~ = exp(logits), d = P~ @ 1):
    g_r = (P~ @ u_r) / d        with u_r = v_r @ w_vd  (folded into V proj)
    sel = softmax_r(g);  out_h = sum_r (sel_r / d) * A~_r

Schedule notes (from perfetto analysis):
- matmul cost ~= out free-dim rows * 0.42ns at full clock; PE must stay
  continuously busy (p-state ramp) and un-gapped.
- DMA descriptor generation on the Sync engine costs ~430ns per
  dma_start, serializing in issue order: big input DMAs go FIRST, small
  bias loads after.
- Cross-head software pipelining: head h+1's logits matmuls + exp run
  inside head h's psa windows so the scalar-engine exp chain (666ns per
  128x512 tile) never gates the PE at head boundaries.
- pss psum bank is drained to SBUF with one vector op so the next
  window's pss accumulation never waits on the selection math.
"""

import numpy as np
from contextlib import ExitStack

import ml_dtypes
import concourse.bass as bass
import concourse.bacc as bacc
import concourse.mybir as mybir
from concourse import tile
from concourse.bass_utils import run_bass_kernel_spmd

AF = mybir.ActivationFunctionType
ALU = mybir.AluOpType
AX = mybir.AxisListType
F32 = mybir.dt.float32

T, B, E, H, R = 2048, 2, 1024, 16, 8
HD, VD, SEL = 64, 128, 64
TS = T // 4            # 512 t-rows per core
NK = E // 128          # 8 contraction chunks over E
NS = T // 128          # 16 s-chunks
NT = TS // 128         # 4 t-chunks per core
VW = R * VD            # 1024 v columns
VX = VW + R            # 1032: v columns + 8 u columns
NCORES = 8

# compute dtype knob: "bf16" | "f32r" | "f32"
COMPUTE = "bf16"

if COMPUTE == "bf16":
    DT = mybir.dt.bfloat16
    NPDT = ml_dtypes.bfloat16
else:
    DT = mybir.dt.float32
    NPDT = np.float32
MM = mybir.dt.float32r if COMPUTE == "f32r" else DT


def _mm(ap):
    """Bitcast an AP to the matmul dtype (float32r only differs in PE mode)."""
    return ap.bitcast(MM) if MM != DT else ap


def _build():
    nc = bacc.Bacc("TRN2", target_bir_lowering=False, debug=False,
                   num_devices=NCORES)
    qt_full = nc.declare_dram_parameter("qt_full", [E, T], DT, isOutput=False)
    qt_slice = nc.declare_dram_parameter("qt_slice", [E, TS], DT, isOutput=False)
    wqt = nc.declare_dram_parameter("wqt", [E, E], DT, isOutput=False)
    # k-projection is sharded across the 4 cores of a batch group and
    # exchanged with one AllGather: every core computes head-pair 0
    # locally (needed before the gather can land) plus its own 2-pair
    # share (pairs 2*hg, 2*hg+1 -- per-core weight slices from the host).
    wk_p0 = nc.declare_dram_parameter("wk_p0", [E, 128], DT, isOutput=False)
    wk_sh = nc.declare_dram_parameter("wk_sh", [E, 256], DT, isOutput=False)
    bk_sh = nc.declare_dram_parameter("bk_sh", [128, 2], F32, isOutput=False)
    # col m = bq chunk m (scaled), col 8+m = bk chunk m
    qk_bias = nc.declare_dram_parameter("qk_bias", [128, 2 * NK], F32,
                                        isOutput=False)
    wvt = nc.declare_dram_parameter("wvt", [E, VX], DT, isOutput=False)
    vbias = nc.declare_dram_parameter("vbias", [128, VX], DT, isOutput=False)
    wot = nc.declare_dram_parameter("wot", [H * VD, E], DT, isOutput=False)
    bo_bc = nc.declare_dram_parameter("bo_bc", [128, E], DT, isOutput=False)
    ident = nc.declare_dram_parameter("ident", [128, 128], DT, isOutput=False)
    out = nc.declare_dram_parameter("out", [TS, E], F32, isOutput=True)
    ag_in = nc.dram_tensor("ag_in", [256, T], DT)
    ag_out = nc.dram_tensor("ag_out", [1024, T], DT)

    with ExitStack() as ctx:
        tc = ctx.enter_context(tile.TileContext(nc))
        pers = ctx.enter_context(tc.tile_pool(name="pers", bufs=1))

        # persistent SBUF tensors
        kt = [pers.tile([128, T], DT, tag=f"kt{m}", name=f"kt{m}") for m in range(NK)]
        # per-head q, zero-padded to the full 128-row head pair so the
        # logits matmul runs with K=128 (FWL engages; K=64 lhsT loads slow)
        qp = [pers.tile([128, TS], DT, tag=f"qp{h}", name=f"qp{h}") for h in range(H)]
        va = [pers.tile([128, VX + 1], DT, tag=f"va{s}", name=f"va{s}") for s in range(NS)]
        # transposed (VD, t) tiles for the out_proj lhsT
        oa = [pers.tile([128, TS], DT, tag=f"oa{h}", name=f"oa{h}") for h in range(H)]
        qk_sb = pers.tile([128, 2 * NK], F32, tag="qk", name="qk_sb")
        vb_sb = pers.tile([128, VX], DT, tag="vb", name="vb")
        bo_sb = pers.tile([128, E], DT, tag="bo", name="bo")
        id_sb = pers.tile([128, 128], DT, tag="id", name="id")

        # logits psum pool lives from the head-0 prologue through phase B
        plt = ctx.enter_context(tc.tile_pool(name="plt", bufs=2, space="PSUM"))
        # exp(logits) tiles, strict ping-pong by head parity; right-side
        # SBUF stack so the pool outlives the left-side phase-A pools
        etp = ctx.enter_context(tc.tile_pool(name="etp", bufs=1, side="right"))
        et = [[etp.tile([128, TS], DT, tag=f"et{p}_{s}", name=f"et{p}_{s}")
               for s in range(NS)] for p in range(2)]

        # ---- Phase A: projections ----
        with tc.tile_pool(name="phQT", bufs=1) as phQT:
            qtf = [phQT.tile([128, T], DT, tag=f"qtf{k}", name=f"qtf{k}") for k in range(NK)]
            with tc.tile_pool(name="ppk", bufs=2, space="PSUM") as ppk:
                with (
                    tc.tile_pool(name="phKW", bufs=1) as phKW,
                    tc.tile_pool(name="phA2", bufs=1) as phA2,
                ):
                    qts = [phA2.tile([128, TS], DT, tag=f"qts{k}", name=f"qts{k}") for k in range(NK)]
                    wq = [phA2.tile([128, E], DT, tag=f"wq{k}", name=f"wq{k}") for k in range(NK)]
                    wkp = [phKW.tile([128, 128], DT, tag=f"wkp{k}", name=f"wkp{k}") for k in range(NK)]
                    wks = [phKW.tile([128, 256], DT, tag=f"wks{k}", name=f"wks{k}") for k in range(NK)]
                    kx = [phKW.tile([128, T], DT, tag=f"kx{j}", name=f"kx{j}") for j in range(2)]
                    bks_sb = phKW.tile([128, 2], F32, tag="bks", name="bks")

                    # DMA order. Sync-engine desc generation is ~430ns
                    # per dma_start and serializes in issue order, so the
                    # q-proj inputs (the first matmuls' deps) go first,
                    # then k-proj inputs, then everything else.
                    for k in range(NK):
                        r0, r1 = k * 128, (k + 1) * 128
                        nc.sync.dma_start(qts[k][:], qt_slice[r0:r1, :])
                        nc.sync.dma_start(wq[k][:, 0:512], wqt[r0:r1, 0:512])
                    for k in range(NK):
                        r0, r1 = k * 128, (k + 1) * 128
                        nc.sync.dma_start(wq[k][:, 512:E], wqt[r0:r1, 512:E])
                    nc.sync.dma_start(qk_sb[:], qk_bias[:])
                    nc.sync.dma_start(bks_sb[:], bk_sh[:])
                    for k in range(NK):
                        r0, r1 = k * 128, (k + 1) * 128
                        nc.sync.dma_start(wkp[k][:], wk_p0[r0:r1, :])
                        nc.sync.dma_start(wks[k][:], wk_sh[r0:r1, :])
                    for c in range(4):
                        for k in range(NK):
                            r0, r1 = k * 128, (k + 1) * 128
                            nc.sync.dma_start(
                                qtf[k][:, c * 512:(c + 1) * 512],
                                qt_full[r0:r1, c * 512:(c + 1) * 512])

                    # q projection -> per-head zero-padded tiles
                    for m in range(NK):
                        c0, c1 = m * 128, (m + 1) * 128
                        ps = ppk.tile([128, 512], F32, tag="ppk", name="ppk")
                        for k in range(NK):
                            nc.tensor.matmul(
                                ps[:], lhsT=_mm(wq[k][:, c0:c1]),
                                rhs=_mm(qts[k][:]),
                                start=(k == 0), stop=(k == NK - 1))
                        h0, h1 = 2 * m, 2 * m + 1
                        nc.vector.memset(qp[h0][64:128, :], 0.0)
                        nc.vector.memset(qp[h1][0:64, :], 0.0)
                        nc.scalar.activation(qp[h0][0:64, :], ps[0:64, :],
                                             AF.Identity,
                                             bias=qk_sb[0:64, m:m + 1])
                        nc.scalar.activation(qp[h1][64:128, :], ps[64:128, :],
                                             AF.Identity,
                                             bias=qk_sb[64:128, m:m + 1])

                    # local k-projection: pair 0 (needed before the gather
                    # lands) into kt[0], plus this core's 2-pair share
                    # into kx; nb-outer so the first sweep only needs qtf
                    # chunk 0 (matches DMA order)
                    for nb in range(T // 512):
                        w0, w1 = nb * 512, (nb + 1) * 512
                        ps = ppk.tile([128, 512], F32, tag="ppk", name="ppk")
                        for k in range(NK):
                            nc.tensor.matmul(
                                ps[:], lhsT=_mm(wkp[k][:]),
                                rhs=_mm(qtf[k][:, w0:w1]),
                                start=(k == 0), stop=(k == NK - 1))
                        nc.scalar.activation(
                            kt[0][:, w0:w1], ps[:], AF.Identity,
                            bias=qk_sb[:, NK:NK + 1])
                        for j in range(2):
                            ps = ppk.tile([128, 512], F32, tag="ppk", name="ppk")
                            for k in range(NK):
                                nc.tensor.matmul(
                                    ps[:],
                                    lhsT=_mm(wks[k][:, j * 128:(j + 1) * 128]),
                                    rhs=_mm(qtf[k][:, w0:w1]),
                                    start=(k == 0), stop=(k == NK - 1))
                            nc.scalar.activation(
                                kx[j][:, w0:w1], ps[:], AF.Identity,
                                bias=bks_sb[:, j:j + 1])

                    # exchange: AllGather the 2-pair shares within each
                    # batch group; output rows are rank-ordered, i.e. the
                    # full kT in global pair order
                    for j in range(2):
                        nc.sync.dma_start(ag_in[j * 128:(j + 1) * 128, :],
                                          kx[j][:])
                    nc.gpsimd.collective_compute(
                        "AllGather",
                        mybir.AluOpType.bypass,
                        replica_groups=[[0, 1, 2, 3], [4, 5, 6, 7]],
                        ins=[ag_in.ap().opt()],
                        outs=[ag_out.ap().opt()],
                    )

            # v_all (s on partitions): [v | u] + bias, plus ones col.
            # Interleaved: head 0's logits+exp (prologue for the
            # cross-head pipeline) run between the vproj psum groups.
            with (
                tc.tile_pool(name="phWV", bufs=1) as phWV,
                tc.tile_pool(name="ppv", bufs=2, space="PSUM") as ppv,
            ):
                wv = [phWV.tile([128, VX], DT, tag=f"wv{k}", name=f"wv{k}") for k in range(NK)]
                for k in range(NK):
                    r0, r1 = k * 128, (k + 1) * 128
                    nc.sync.dma_start(wv[k][:, 0:516], wvt[r0:r1, 0:516])
                    nc.sync.dma_start(wv[k][:, 516:VX], wvt[r0:r1, 516:VX])
                nc.sync.dma_start(vb_sb[:], vbias[:])
                nc.sync.dma_start(bo_sb[:], bo_bc[:])
                nc.sync.dma_start(id_sb[:], ident[:])
                for s in range(NS):
                    c0, c1 = s * 128, (s + 1) * 128
                    psv = ppv.tile([128, VX], F32, tag="ppv", name="ppv")
                    for k in range(NK):
                        lhs = _mm(qtf[k][:, c0:c1])
                        nc.tensor.matmul(psv[:, 0:512], lhsT=lhs,
                                         rhs=_mm(wv[k][:, 0:512]),
                                         start=(k == 0), stop=(k == NK - 1))
                        nc.tensor.matmul(psv[:, 512:1024], lhsT=lhs,
                                         rhs=_mm(wv[k][:, 512:1024]),
                                         start=(k == 0), stop=(k == NK - 1))
                        nc.tensor.matmul(psv[:, 1024:VX], lhsT=lhs,
                                         rhs=_mm(wv[k][:, 1024:VX]),
                                         start=(k == 0), stop=(k == NK - 1))
                    nc.vector.tensor_add(va[s][:, 0:VX], psv[:], vb_sb[:])
                    nc.vector.memset(va[s][:, VX:VX + 1], 1.0)
                    # head-0 logits for this s-chunk
                    psl = plt.tile([128, TS], F32, tag="plt", name="psl")
                    nc.tensor.matmul(
                        psl[:], lhsT=_mm(kt[0][:, c0:c1]), rhs=_mm(qp[0][:]),
                        start=True, stop=True)
                    nc.scalar.activation(et[0][s][:], psl[:], AF.Exp)

        # ---- load Wo^T into space freed by phase A ----
        woPool = ctx.enter_context(tc.tile_pool(name="wo", bufs=1))
        wo = [woPool.tile([128, E], DT, tag=f"wo{k}", name=f"wo{k}") for k in range(H)]
        for k in range(H):
            nc.sync.dma_start(wo[k][:], wot[k * 128:(k + 1) * 128, :])
        # gathered k pairs 1..7, in first-consumer order given HEAD_ORDER;
        # issued after the wo DMAs so queue slots waiting on the gather
        # semaphore never block earlier transfers
        for m in (2, 3, 4, 5, 6, 7, 1):
            nc.sync.dma_start(kt[m][:], ag_out[m * 128:(m + 1) * 128, :])

        # ---- Phase B: attention per head, cross-head pipelined ----
        with (
            tc.tile_pool(name="pa", bufs=2, space="PSUM") as pa,
            tc.tile_pool(name="pas", bufs=1, space="PSUM") as pas,
            tc.tile_pool(name="pt", bufs=1, space="PSUM") as pt,
            tc.tile_pool(name="sm", bufs=4) as sm,
            tc.tile_pool(name="ocp", bufs=4) as ocp,
            tc.tile_pool(name="ob", bufs=2) as obp,
        ):
            def issue_transpose(hh, tt, octile):
                ptr = pt.tile([128, 128], DT, tag="ptr", name="ptr")
                nc.tensor.transpose(ptr[:], octile[:], id_sb[:])
                nc.vector.tensor_scalar_mul(
                    oa[hh][:, tt * 128:(tt + 1) * 128], ptr[:], 1.0)

            # out-proj psums come from the always-open plt pool so the
            # epilogue never waits on a PSUM pool transition; chunks for
            # early t are interleaved into the last head's windows
            def outproj_chunk(t):
                t0, t1 = t * 128, (t + 1) * 128
                for e in range(E // 512):
                    pso = plt.tile([128, TS], F32, tag="plt", name="pso")
                    for k in range(H):
                        nc.tensor.matmul(
                            pso[:], lhsT=_mm(oa[k][:, t0:t1]),
                            rhs=_mm(wo[k][:, e * 512:(e + 1) * 512]),
                            start=(k == 0), stop=(k == H - 1))
                    ob = obp.tile([128, 512], F32, tag="ob", name="ob")
                    nc.vector.tensor_add(ob[:], pso[:, 0:512],
                                         bo_sb[:, e * 512:(e + 1) * 512])
                    nc.sync.dma_start(out[t0:t1, e * 512:(e + 1) * 512],
                                      ob[:])

            # pair-0 heads first (computed locally), pair-1 heads (2,3)
            # LAST: their kt arrives from the AllGather with the most
            # slack; pairs 2..7 are consumed in gather-writeback order
            HEAD_ORDER = [0, 1] + list(range(4, H)) + [2, 3]
            pending = []
            for v in range(H):
                h = HEAD_ORDER[v]
                cur = et[v % 2]
                nxt = et[(v + 1) % 2]
                for t in range(NT):
                    t0, t1 = t * 128, (t + 1) * 128
                    psa = pa.tile([128, VW], F32, tag="psa", name="psa")
                    pss = pas.tile([128, R + 1], F32, tag="pss", name="pss")
                    for s in range(NS):
                        lhs = _mm(cur[s][:, t0:t1])
                        st, sp = (s == 0), (s == NS - 1)
                        nc.tensor.matmul(psa[:, 0:512], lhsT=lhs,
                                         rhs=_mm(va[s][:, 0:512]),
                                         start=st, stop=sp)
                        nc.tensor.matmul(psa[:, 512:1024], lhsT=lhs,
                                         rhs=_mm(va[s][:, 512:1024]),
                                         start=st, stop=sp)
                        nc.tensor.matmul(pss[:], lhsT=lhs,
                                         rhs=_mm(va[s][:, 1024:VX + 1]),
                                         start=st, stop=sp)
                        # next head's logits, one per 4 s-steps (spread so
                        # the in-order PE never waits on the plt->exp
                        # drain): the next head finds all its exp tiles
                        # ready at its boundary
                        if v + 1 < H and s % 4 == 3:
                            hn = HEAD_ORDER[v + 1]
                            s4 = t * 4 + s // 4
                            psl = plt.tile([128, TS], F32, tag="plt", name="psl")
                            nc.tensor.matmul(
                                psl[:],
                                lhsT=_mm(kt[hn // 2][:, s4 * 128:(s4 + 1) * 128]),
                                rhs=_mm(qp[hn][:]),
                                start=True, stop=True)
                            nc.scalar.activation(nxt[s4][:], psl[:], AF.Exp)
                    # drain pss to SBUF in one op so the next window's
                    # accumulation never waits on the selection math
                    pssS = sm.tile([128, R + 1], F32, tag="pssS", name="pssS")
                    nc.vector.tensor_scalar_mul(pssS[:], pss[:], 1.0)
                    # selection weights: w_r = softmax_r(G~_r/d) / d
                    rcp_d = sm.tile([128, 1], F32, tag="rcpd", name="rcpd")
                    nc.vector.reciprocal(rcp_d[:], pssS[:, R:R + 1])
                    g = sm.tile([128, R], F32, tag="g", name="g")
                    nc.vector.tensor_scalar_mul(g[:], pssS[:, 0:R], rcp_d[:])
                    selw = sm.tile([128, R], F32, tag="selw", name="selw")
                    nc.scalar.activation(selw[:], g[:], AF.Exp)
                    ssum = sm.tile([128, 1], F32, tag="ssum", name="ssum")
                    nc.vector.tensor_reduce(ssum[:], selw[:], AX.XYZW, ALU.add)
                    den = sm.tile([128, 1], F32, tag="den", name="den")
                    nc.vector.tensor_scalar_mul(den[:], ssum[:],
                                                pssS[:, R:R + 1])
                    rcp2 = sm.tile([128, 1], F32, tag="rcp2", name="rcp2")
                    nc.vector.reciprocal(rcp2[:], den[:])
                    w = sm.tile([128, R], F32, tag="w", name="w")
                    nc.vector.tensor_scalar_mul(w[:], selw[:], rcp2[:])
                    # combine rules: out_tile = sum_r w_r * A~_r
                    acc = sm.tile([128, 128], F32, tag="acc", name="acc")
                    nc.vector.tensor_scalar_mul(acc[:], psa[:, 0:128],
                                                w[:, 0:1])
                    for r in range(1, R - 1):
                        acc2 = sm.tile([128, 128], F32, tag="acc", name="acc")
                        nc.vector.scalar_tensor_tensor(
                            acc2[:], psa[:, r * 128:(r + 1) * 128],
                            w[:, r:r + 1], acc[:],
                            op0=ALU.mult, op1=ALU.add)
                        acc = acc2
                    octile = ocp.tile([128, VD], DT, tag="oc", name="oc")
                    nc.vector.scalar_tensor_tensor(
                        octile[:], psa[:, (R - 1) * 128:R * 128],
                        w[:, R - 1:R], acc[:], op0=ALU.mult, op1=ALU.add)
                    if pending:
                        issue_transpose(*pending.pop(0))
                    pending.append((h, t, octile))
                    # last head: stream out-proj chunks as their oa
                    # columns complete (copy of (last,t-2) lands early in
                    # this window's psa)
                    if v == H - 1 and t >= 2:
                        outproj_chunk(t - 2)
            outproj_chunk(NT - 2)
            for hh, tt, octile in pending:
                issue_transpose(hh, tt, octile)
            outproj_chunk(NT - 1)
    nc.finalize()
    return nc


_NC_CACHE = None


def _get_nc():
    global _NC_CACHE
    if _NC_CACHE is None:
        _NC_CACHE = _build()
    return _NC_CACHE


def _prep_in_maps(query, Wq, bq, Wk, bk, Wv, bv, Wsc, Wo, bo):
    scale = np.float32(HD ** -0.5)
    w_vd = Wsc[0, SEL:].astype(np.float32)          # (VD,)

    wqt = np.ascontiguousarray((Wq * scale).T).astype(NPDT)
    wkt = np.ascontiguousarray(Wk.T).astype(NPDT)
    bk_cols = bk.reshape(NK, 128).T.astype(np.float32)       # (128, NK)
    qk = np.empty((128, 2 * NK), np.float32)
    qk[:, 0:NK] = (bq * scale).reshape(NK, 128).T
    qk[:, NK:2 * NK] = bk_cols
    wk_p0 = np.ascontiguousarray(wkt[:, 0:128])

    WvT = np.ascontiguousarray(Wv.T).astype(np.float32)      # (E, VW)
    U_w = np.einsum("erd,d->er", WvT.reshape(E, R, VD), w_vd)  # (E, R)
    wvt = np.concatenate([WvT, U_w], axis=1).astype(NPDT)    # (E, VX)
    ubias = np.einsum("rd,d->r", bv.reshape(R, VD), w_vd)    # (R,)
    vb_row = np.concatenate([bv.astype(np.float32), ubias.astype(np.float32)])
    vbias = np.ascontiguousarray(
        np.broadcast_to(vb_row, (128, VX))).astype(NPDT)

    wot = np.ascontiguousarray(Wo.T).astype(NPDT)            # (H*VD, E)
    bo_bc = np.ascontiguousarray(
        np.broadcast_to(bo, (128, E))).astype(NPDT)
    ident = np.eye(128, dtype=NPDT)

    shared = dict(wqt=wqt, wk_p0=wk_p0, qk_bias=qk, wvt=wvt,
                  vbias=vbias, wot=wot, bo_bc=bo_bc, ident=ident)

    in_maps = []
    for c in range(NCORES):
        b, tq = c // 4, c % 4
        qT = np.ascontiguousarray(query[:, b, :].T).astype(NPDT)  # (E, T)
        m = dict(shared)
        m["qt_full"] = qT
        m["qt_slice"] = np.ascontiguousarray(qT[:, tq * TS:(tq + 1) * TS])
        # this core's k-projection share: head-pairs (2*tq, 2*tq+1)
        m["wk_sh"] = np.ascontiguousarray(wkt[:, tq * 256:(tq + 1) * 256])
        m["bk_sh"] = np.ascontiguousarray(bk_cols[:, 2 * tq:2 * tq + 2])
        in_maps.append(m)
    return in_maps


def kernel(query, Wq, bq, Wk, bk, Wv, bv, Wvq, bvq, Wsc, bsc, Wo, bo,
           _trace=False, _tmpdir=None):
    query = np.asarray(query, np.float32)
    in_maps = _prep_in_maps(
        np.asarray(query, np.float32), np.asarray(Wq, np.float32),
        np.asarray(bq, np.float32), np.asarray(Wk, np.float32),
        np.asarray(bk, np.float32), np.asarray(Wv, np.float32),
        np.asarray(bv, np.float32), np.asarray(Wsc, np.float32),
        np.asarray(Wo, np.float32), np.asarray(bo, np.float32))
    nc = _get_nc()
    res = run_bass_kernel_spmd(nc, in_maps, list(range(NCORES)),
                               trace=_trace, tmpdir=_tmpdir)
    out = np.empty((T, B, E), np.float32)
    for c in range(NCORES):
        b, tq = c // 4, c % 4
        out[tq * TS:(tq + 1) * TS, b, :] = res.results[c]["out"]
    kernel._last_results = res
    return out


# revision 27
# speedup vs baseline: 1.1326x; 1.1326x over previous
"""CompositionalAttention Trainium2 kernel (8 NeuronCores, SPMD).

Shapes (hardcoded): query (T=2048, B=2, E=1024), H=16 heads, R=8 rules,
HD=64, VD=128. Output (T, B, E) float32.

Sharding: (batch x t-quarter) -> 8 cores. Core c handles b = c//4 and the
t-slice [tq*512, (tq+1)*512) with tq = c%4, computing ALL heads for that
slice so the output projection needs no cross-core reduction. Each core
returns its exact (512, 1024) slice of the final output.

Algebraic simplification used (verified vs reference to 2.5e-6):
the rule-selection softmax input is
    score[b,h,t,r] = v_q . w_sel + bsc + attn[b,h,t,r,:] . w_vd
and the first two terms are constant in r, so they cancel in the softmax
over r. Wvq/bvq/Wsc[:, :SEL]/bsc never affect the output. Further, with
unnormalized attention A~_r = P~ @ v_r (P~ = exp(logits), d = P~ @ 1):
    g_r = (P~ @ u_r) / d        with u_r = v_r @ w_vd  (folded into V proj)
    sel = softmax_r(g);  out_h = sum_r (sel_r / d) * A~_r

Schedule notes (from perfetto analysis):
- matmul cost ~= out free-dim rows * 0.42ns at full clock; PE must stay
  continuously busy (p-state ramp) and un-gapped.
- DMA descriptor generation on the Sync engine costs ~430ns per
  dma_start, serializing in issue order: big input DMAs go FIRST, small
  bias loads after.
- Cross-head software pipelining: head h+1's logits matmuls + exp run
  inside head h's psa windows so the scalar-engine exp chain (666ns per
  128x512 tile) never gates the PE at head boundaries.
- pss psum bank is drained to SBUF with one vector op so the next
  window's pss accumulation never waits on the selection math.
"""

import numpy as np
from contextlib import ExitStack

import ml_dtypes
import concourse.bass as bass
import concourse.bacc as bacc
import concourse.mybir as mybir
from concourse import tile
from concourse.bass_utils import run_bass_kernel_spmd

AF = mybir.ActivationFunctionType
ALU = mybir.AluOpType
AX = mybir.AxisListType
F32 = mybir.dt.float32

T, B, E, H, R = 2048, 2, 1024, 16, 8
HD, VD, SEL = 64, 128, 64
TS = T // 4            # 512 t-rows per core
NK = E // 128          # 8 contraction chunks over E
NS = T // 128          # 16 s-chunks
NT = TS // 128         # 4 t-chunks per core
VW = R * VD            # 1024 v columns
VX = VW + R            # 1032: v columns + 8 u columns
NCORES = 8

# compute dtype knob: "bf16" | "f32r" | "f32"
COMPUTE = "bf16"

if COMPUTE == "bf16":
    DT = mybir.dt.bfloat16
    NPDT = ml_dtypes.bfloat16
else:
    DT = mybir.dt.float32
    NPDT = np.float32
MM = mybir.dt.float32r if COMPUTE == "f32r" else DT


def _mm(ap):
    """Bitcast an AP to the matmul dtype (float32r only differs in PE mode)."""
    return ap.bitcast(MM) if MM != DT else ap


def _build():
    nc = bacc.Bacc("TRN2", target_bir_lowering=False, debug=False,
                   num_devices=NCORES)
    qt_full = nc.declare_dram_parameter("qt_full", [E, T], DT, isOutput=False)
    qt_slice = nc.declare_dram_parameter("qt_slice", [E, TS], DT, isOutput=False)
    wqt = nc.declare_dram_parameter("wqt", [E, E], DT, isOutput=False)
    wkt = nc.declare_dram_parameter("wkt", [E, E], DT, isOutput=False)
    # col m = bq chunk m (scaled), col 8+m = bk chunk m
    qk_bias = nc.declare_dram_parameter("qk_bias", [128, 2 * NK], F32,
                                        isOutput=False)
    wvt = nc.declare_dram_parameter("wvt", [E, VX], DT, isOutput=False)
    vbias = nc.declare_dram_parameter("vbias", [128, VX], DT, isOutput=False)
    wot = nc.declare_dram_parameter("wot", [H * VD, E], DT, isOutput=False)
    bo_bc = nc.declare_dram_parameter("bo_bc", [128, E], DT, isOutput=False)
    ident = nc.declare_dram_parameter("ident", [128, 128], DT, isOutput=False)
    out = nc.declare_dram_parameter("out", [TS, E], F32, isOutput=True)

    with ExitStack() as ctx:
        tc = ctx.enter_context(tile.TileContext(nc))
        pers = ctx.enter_context(tc.tile_pool(name="pers", bufs=1))

        # persistent SBUF tensors
        kt = [pers.tile([128, T], DT, tag=f"kt{m}", name=f"kt{m}") for m in range(NK)]
        # per-head q, zero-padded to the full 128-row head pair so the
        # logits matmul runs with K=128 (FWL engages; K=64 lhsT loads slow)
        qp = [pers.tile([128, TS], DT, tag=f"qp{h}", name=f"qp{h}") for h in range(H)]
        va = [pers.tile([128, VX + 1], DT, tag=f"va{s}", name=f"va{s}") for s in range(NS)]
        # transposed (VD, t) tiles for the out_proj lhsT
        oa = [pers.tile([128, TS], DT, tag=f"oa{h}", name=f"oa{h}") for h in range(H)]
        qk_sb = pers.tile([128, 2 * NK], F32, tag="qk", name="qk_sb")
        vb_sb = pers.tile([128, VX], DT, tag="vb", name="vb")
        bo_sb = pers.tile([128, E], DT, tag="bo", name="bo")
        id_sb = pers.tile([128, 128], DT, tag="id", name="id")

        # logits psum pool lives from the head-0 prologue through phase B
        plt = ctx.enter_context(tc.tile_pool(name="plt", bufs=2, space="PSUM"))
        # exp(logits) tiles, strict ping-pong by head parity; right-side
        # SBUF stack so the pool outlives the left-side phase-A pools
        etp = ctx.enter_context(tc.tile_pool(name="etp", bufs=1, side="right"))
        et = [[etp.tile([128, TS], DT, tag=f"et{p}_{s}", name=f"et{p}_{s}")
               for s in range(NS)] for p in range(2)]

        # ---- Phase A: projections ----
        with tc.tile_pool(name="phQT", bufs=1) as phQT:
            qtf = [phQT.tile([128, T], DT, tag=f"qtf{k}", name=f"qtf{k}") for k in range(NK)]
            with tc.tile_pool(name="ppk", bufs=2, space="PSUM") as ppk:
                with (
                    tc.tile_pool(name="phKW", bufs=1) as phKW,
                    tc.tile_pool(name="phA2", bufs=1) as phA2,
                ):
                    qts = [phA2.tile([128, TS], DT, tag=f"qts{k}", name=f"qts{k}") for k in range(NK)]
                    wq = [phA2.tile([128, E], DT, tag=f"wq{k}", name=f"wq{k}") for k in range(NK)]
                    wk = [phKW.tile([128, E], DT, tag=f"wk{k}", name=f"wk{k}") for k in range(NK)]

                    # DMA order. Sync-engine desc generation is ~430ns
                    # per dma_start and serializes in issue order, so the
                    # q-proj inputs (the first matmuls' deps) go first,
                    # then k-proj inputs, then everything else.
                    for k in range(NK):
                        r0, r1 = k * 128, (k + 1) * 128
                        nc.sync.dma_start(qts[k][:], qt_slice[r0:r1, :])
                        nc.sync.dma_start(wq[k][:, 0:512], wqt[r0:r1, 0:512])
                    for k in range(NK):
                        r0, r1 = k * 128, (k + 1) * 128
                        nc.sync.dma_start(wq[k][:, 512:E], wqt[r0:r1, 512:E])
                    nc.sync.dma_start(qk_sb[:], qk_bias[:])
                    for k in range(NK):
                        r0, r1 = k * 128, (k + 1) * 128
                        nc.sync.dma_start(wk[k][:, 0:512], wkt[r0:r1, 0:512])
                    for k in range(NK):
                        r0, r1 = k * 128, (k + 1) * 128
                        nc.sync.dma_start(qtf[k][:, 0:512],
                                          qt_full[r0:r1, 0:512])
                    for k in range(NK):
                        r0, r1 = k * 128, (k + 1) * 128
                        nc.sync.dma_start(wk[k][:, 512:E], wkt[r0:r1, 512:E])
                    for c in range(1, 4):
                        for k in range(NK):
                            r0, r1 = k * 128, (k + 1) * 128
                            nc.sync.dma_start(
                                qtf[k][:, c * 512:(c + 1) * 512],
                                qt_full[r0:r1, c * 512:(c + 1) * 512])

                    # q projection -> per-head zero-padded tiles
                    for m in range(NK):
                        c0, c1 = m * 128, (m + 1) * 128
                        ps = ppk.tile([128, 512], F32, tag="ppk", name="ppk")
                        for k in range(NK):
                            nc.tensor.matmul(
                                ps[:], lhsT=_mm(wq[k][:, c0:c1]),
                                rhs=_mm(qts[k][:]),
                                start=(k == 0), stop=(k == NK - 1))
                        h0, h1 = 2 * m, 2 * m + 1
                        nc.vector.memset(qp[h0][64:128, :], 0.0)
                        nc.vector.memset(qp[h1][0:64, :], 0.0)
                        nc.scalar.activation(qp[h0][0:64, :], ps[0:64, :],
                                             AF.Identity,
                                             bias=qk_sb[0:64, m:m + 1])
                        nc.scalar.activation(qp[h1][64:128, :], ps[64:128, :],
                                             AF.Identity,
                                             bias=qk_sb[64:128, m:m + 1])

                    # kT_all (E_out on partitions, s free); nb-outer so the
                    # first sweep only needs qtf chunk 0 (matches DMA order)
                    for nb in range(T // 512):
                        for m in range(NK):
                            c0, c1 = m * 128, (m + 1) * 128
                            ps = ppk.tile([128, 512], F32, tag="ppk", name="ppk")
                            for k in range(NK):
                                nc.tensor.matmul(
                                    ps[:], lhsT=_mm(wk[k][:, c0:c1]),
                                    rhs=_mm(qtf[k][:, nb * 512:(nb + 1) * 512]),
                                    start=(k == 0), stop=(k == NK - 1))
                            nc.scalar.activation(
                                kt[m][:, nb * 512:(nb + 1) * 512],
                                ps[:], AF.Identity,
                                bias=qk_sb[:, NK + m:NK + m + 1])

            # v_all (s on partitions): [v | u] + bias, plus ones col.
            # Interleaved: head 0's logits+exp (prologue for the
            # cross-head pipeline) run between the vproj psum groups.
            with (
                tc.tile_pool(name="phWV", bufs=1) as phWV,
                tc.tile_pool(name="ppv", bufs=2, space="PSUM") as ppv,
            ):
                wv = [phWV.tile([128, VX], DT, tag=f"wv{k}", name=f"wv{k}") for k in range(NK)]
                for k in range(NK):
                    r0, r1 = k * 128, (k + 1) * 128
                    nc.sync.dma_start(wv[k][:, 0:516], wvt[r0:r1, 0:516])
                    nc.sync.dma_start(wv[k][:, 516:VX], wvt[r0:r1, 516:VX])
                nc.sync.dma_start(vb_sb[:], vbias[:])
                nc.sync.dma_start(bo_sb[:], bo_bc[:])
                nc.sync.dma_start(id_sb[:], ident[:])
                for s in range(NS):
                    c0, c1 = s * 128, (s + 1) * 128
                    psv = ppv.tile([128, VX], F32, tag="ppv", name="ppv")
                    for k in range(NK):
                        lhs = _mm(qtf[k][:, c0:c1])
                        nc.tensor.matmul(psv[:, 0:512], lhsT=lhs,
                                         rhs=_mm(wv[k][:, 0:512]),
                                         start=(k == 0), stop=(k == NK - 1))
                        nc.tensor.matmul(psv[:, 512:1024], lhsT=lhs,
                                         rhs=_mm(wv[k][:, 512:1024]),
                                         start=(k == 0), stop=(k == NK - 1))
                        nc.tensor.matmul(psv[:, 1024:VX], lhsT=lhs,
                                         rhs=_mm(wv[k][:, 1024:VX]),
                                         start=(k == 0), stop=(k == NK - 1))
                    nc.vector.tensor_add(va[s][:, 0:VX], psv[:], vb_sb[:])
                    nc.vector.memset(va[s][:, VX:VX + 1], 1.0)
                    # head-0 logits for this s-chunk
                    psl = plt.tile([128, TS], F32, tag="plt", name="psl")
                    nc.tensor.matmul(
                        psl[:], lhsT=_mm(kt[0][:, c0:c1]), rhs=_mm(qp[0][:]),
                        start=True, stop=True)
                    nc.scalar.activation(et[0][s][:], psl[:], AF.Exp)

        # ---- load Wo^T into space freed by phase A ----
        woPool = ctx.enter_context(tc.tile_pool(name="wo", bufs=1))
        wo = [woPool.tile([128, E], DT, tag=f"wo{k}", name=f"wo{k}") for k in range(H)]
        for k in range(H):
            nc.sync.dma_start(wo[k][:], wot[k * 128:(k + 1) * 128, :])

        # ---- Phase B: attention per head, cross-head pipelined ----
        with (
            tc.tile_pool(name="pa", bufs=2, space="PSUM") as pa,
            tc.tile_pool(name="pas", bufs=1, space="PSUM") as pas,
            tc.tile_pool(name="pt", bufs=1, space="PSUM") as pt,
            tc.tile_pool(name="sm", bufs=4) as sm,
            tc.tile_pool(name="ocp", bufs=4) as ocp,
            tc.tile_pool(name="ob", bufs=2) as obp,
        ):
            def issue_transpose(hh, tt, octile):
                ptr = pt.tile([128, 128], DT, tag="ptr", name="ptr")
                nc.tensor.transpose(ptr[:], octile[:], id_sb[:])
                nc.vector.tensor_scalar_mul(
                    oa[hh][:, tt * 128:(tt + 1) * 128], ptr[:], 1.0)

            # out-proj psums come from the always-open plt pool so the
            # epilogue never waits on a PSUM pool transition; chunks for
            # early t are interleaved into the last head's windows
            def outproj_chunk(t):
                t0, t1 = t * 128, (t + 1) * 128
                for e in range(E // 512):
                    pso = plt.tile([128, TS], F32, tag="plt", name="pso")
                    for k in range(H):
                        nc.tensor.matmul(
                            pso[:], lhsT=_mm(oa[k][:, t0:t1]),
                            rhs=_mm(wo[k][:, e * 512:(e + 1) * 512]),
                            start=(k == 0), stop=(k == H - 1))
                    ob = obp.tile([128, 512], F32, tag="ob", name="ob")
                    nc.vector.tensor_add(ob[:], pso[:, 0:512],
                                         bo_sb[:, e * 512:(e + 1) * 512])
                    nc.sync.dma_start(out[t0:t1, e * 512:(e + 1) * 512],
                                      ob[:])

            pending = []
            for h in range(H):
                cur = et[h % 2]
                nxt = et[(h + 1) % 2]
                for t in range(NT):
                    t0, t1 = t * 128, (t + 1) * 128
                    psa = pa.tile([128, VW], F32, tag="psa", name="psa")
                    pss = pas.tile([128, R + 1], F32, tag="pss", name="pss")
                    for s in range(NS):
                        lhs = _mm(cur[s][:, t0:t1])
                        st, sp = (s == 0), (s == NS - 1)
                        nc.tensor.matmul(psa[:, 0:512], lhsT=lhs,
                                         rhs=_mm(va[s][:, 0:512]),
                                         start=st, stop=sp)
                        nc.tensor.matmul(psa[:, 512:1024], lhsT=lhs,
                                         rhs=_mm(va[s][:, 512:1024]),
                                         start=st, stop=sp)
                        nc.tensor.matmul(pss[:], lhsT=lhs,
                                         rhs=_mm(va[s][:, 1024:VX + 1]),
                                         start=st, stop=sp)
                        # next head's logits, one per 4 s-steps (spread so
                        # the in-order PE never waits on the plt->exp
                        # drain): head h+1 finds all its exp tiles ready
                        if h + 1 < H and s % 4 == 3:
                            s4 = t * 4 + s // 4
                            psl = plt.tile([128, TS], F32, tag="plt", name="psl")
                            nc.tensor.matmul(
                                psl[:],
                                lhsT=_mm(kt[(h + 1) // 2][:, s4 * 128:(s4 + 1) * 128]),
                                rhs=_mm(qp[h + 1][:]),
                                start=True, stop=True)
                            nc.scalar.activation(nxt[s4][:], psl[:], AF.Exp)
                    # drain pss to SBUF in one op so the next window's
                    # accumulation never waits on the selection math
                    pssS = sm.tile([128, R + 1], F32, tag="pssS", name="pssS")
                    nc.vector.tensor_scalar_mul(pssS[:], pss[:], 1.0)
                    # selection weights: w_r = softmax_r(G~_r/d) / d
                    rcp_d = sm.tile([128, 1], F32, tag="rcpd", name="rcpd")
                    nc.vector.reciprocal(rcp_d[:], pssS[:, R:R + 1])
                    g = sm.tile([128, R], F32, tag="g", name="g")
                    nc.vector.tensor_scalar_mul(g[:], pssS[:, 0:R], rcp_d[:])
                    selw = sm.tile([128, R], F32, tag="selw", name="selw")
                    nc.scalar.activation(selw[:], g[:], AF.Exp)
                    ssum = sm.tile([128, 1], F32, tag="ssum", name="ssum")
                    nc.vector.tensor_reduce(ssum[:], selw[:], AX.XYZW, ALU.add)
                    den = sm.tile([128, 1], F32, tag="den", name="den")
                    nc.vector.tensor_scalar_mul(den[:], ssum[:],
                                                pssS[:, R:R + 1])
                    rcp2 = sm.tile([128, 1], F32, tag="rcp2", name="rcp2")
                    nc.vector.reciprocal(rcp2[:], den[:])
                    w = sm.tile([128, R], F32, tag="w", name="w")
                    nc.vector.tensor_scalar_mul(w[:], selw[:], rcp2[:])
                    # combine rules: out_tile = sum_r w_r * A~_r
                    acc = sm.tile([128, 128], F32, tag="acc", name="acc")
                    nc.vector.tensor_scalar_mul(acc[:], psa[:, 0:128],
                                                w[:, 0:1])
                    for r in range(1, R - 1):
                        acc2 = sm.tile([128, 128], F32, tag="acc", name="acc")
                        nc.vector.scalar_tensor_tensor(
                            acc2[:], psa[:, r * 128:(r + 1) * 128],
                            w[:, r:r + 1], acc[:],
                            op0=ALU.mult, op1=ALU.add)
                        acc = acc2
                    octile = ocp.tile([128, VD], DT, tag="oc", name="oc")
                    nc.vector.scalar_tensor_tensor(
                        octile[:], psa[:, (R - 1) * 128:R * 128],
                        w[:, R - 1:R], acc[:], op0=ALU.mult, op1=ALU.add)
                    if pending:
                        issue_transpose(*pending.pop(0))
                    pending.append((h, t, octile))
                    # last head: stream out-proj chunks as their oa
                    # columns complete (copy of (15,t-2) lands early in
                    # this window's psa)
                    if h == H - 1 and t >= 2:
                        outproj_chunk(t - 2)
            outproj_chunk(NT - 2)
            for hh, tt, octile in pending:
                issue_transpose(hh, tt, octile)
            outproj_chunk(NT - 1)
    nc.finalize()
    return nc


_NC_CACHE = None


def _get_nc():
    global _NC_CACHE
    if _NC_CACHE is None:
        _NC_CACHE = _build()
    return _NC_CACHE


def _prep_in_maps(query, Wq, bq, Wk, bk, Wv, bv, Wsc, Wo, bo):
    scale = np.float32(HD ** -0.5)
    w_vd = Wsc[0, SEL:].astype(np.float32)          # (VD,)

    wqt = np.ascontiguousarray((Wq * scale).T).astype(NPDT)
    wkt = np.ascontiguousarray(Wk.T).astype(NPDT)
    qk = np.empty((128, 2 * NK), np.float32)
    qk[:, 0:NK] = (bq * scale).reshape(NK, 128).T
    qk[:, NK:2 * NK] = bk.reshape(NK, 128).T

    WvT = np.ascontiguousarray(Wv.T).astype(np.float32)      # (E, VW)
    U_w = np.einsum("erd,d->er", WvT.reshape(E, R, VD), w_vd)  # (E, R)
    wvt = np.concatenate([WvT, U_w], axis=1).astype(NPDT)    # (E, VX)
    ubias = np.einsum("rd,d->r", bv.reshape(R, VD), w_vd)    # (R,)
    vb_row = np.concatenate([bv.astype(np.float32), ubias.astype(np.float32)])
    vbias = np.ascontiguousarray(
        np.broadcast_to(vb_row, (128, VX))).astype(NPDT)

    wot = np.ascontiguousarray(Wo.T).astype(NPDT)            # (H*VD, E)
    bo_bc = np.ascontiguousarray(
        np.broadcast_to(bo, (128, E))).astype(NPDT)
    ident = np.eye(128, dtype=NPDT)

    shared = dict(wqt=wqt, wkt=wkt, qk_bias=qk, wvt=wvt,
                  vbias=vbias, wot=wot, bo_bc=bo_bc, ident=ident)

    in_maps = []
    for c in range(NCORES):
        b, tq = c // 4, c % 4
        qT = np.ascontiguousarray(query[:, b, :].T).astype(NPDT)  # (E, T)
        m = dict(shared)
        m["qt_full"] = qT
        m["qt_slice"] = np.ascontiguousarray(qT[:, tq * TS:(tq + 1) * TS])
        in_maps.append(m)
    return in_maps


def kernel(query, Wq, bq, Wk, bk, Wv, bv, Wvq, bvq, Wsc, bsc, Wo, bo,
           _trace=False, _tmpdir=None):
    query = np.asarray(query, np.float32)
    in_maps = _prep_in_maps(
        np.asarray(query, np.float32), np.asarray(Wq, np.float32),
        np.asarray(bq, np.float32), np.asarray(Wk, np.float32),
        np.asarray(bk, np.float32), np.asarray(Wv, np.float32),
        np.asarray(bv, np.float32), np.asarray(Wsc, np.float32),
        np.asarray(Wo, np.float32), np.asarray(bo, np.float32))
    nc = _get_nc()
    res = run_bass_kernel_spmd(nc, in_maps, list(range(NCORES)),
                               trace=_trace, tmpdir=_tmpdir)
    out = np.empty((T, B, E), np.float32)
    for c in range(NCORES):
        b, tq = c // 4, c % 4
        out[tq * TS:(tq + 1) * TS, b, :] = res.results[c]["out"]
    kernel._last_results = res
    return out
